# revision 67
# baseline (speedup 1.0000x reference)
"""Trainium2 Bass kernel for nn_DepthWiseSepConv (depthwise 5x5 + BN+hardswish
+ pointwise 1x1 + squeeze-excite gating + BN), data-parallel over batch on
8 NeuronCores.

Self-contained: hardcodes all shapes from the problem spec.

Per-core layout (B_loc = 8 images per core), all matmul operands fp16:

  - Depthwise conv, operand-swapped Toeplitz form: stationary lhsT = x
    [(2ch x 28h_in) + ones-row, (4b x 28w_out)], moving rhs = per-group
    block-diagonal Toeplitz [(2ch x 28h_in) + bias-row, (2ch x 28h_out)].
    Output lands pixel-major [(32w, 4b), (c4, h)] so only ONE transpose
    stage is needed (to channel-major) before the pointwise conv.  The x
    arena is packed (w-major, b-inner) so the stationary operand's free
    dims merge to a single walrus-legal dimension.  BN1 scale is folded
    into the Toeplitz weights; BN1 bias (+3 for the hardswish trick) rides
    an extra ones-row of x (dx=0 slice only).  5 dx shifts accumulate in
    PSUM.  Channel blocks sit at partition bases 0 and 64 (tile_position
    rule); the first rotation of Toeplitz slots is filled full-pad (zeros
    baked in DRAM), later refills overwrite only the payload columns.
  - Hardswish epilogue: ACT relu (PSUM->f16), DVE min, DVE (a-3)*m -> D.
  - One transpose stage (T2): [(4b,w), (g,c)@h] -> channel-major R tiles.
  - SE: per-image DVE reduces on R, two small matmuls, hardswish; gate and
    BN2 are folded into per-(image,half) ACT scale/bias vectors.
  - Pointwise: [120c x 120o] fp16 matmuls, N=392, single fused epilogue op.
"""

import sys

sys.path.insert(0, "/opt/trn_rl_repo")

import numpy as np

import concourse.bass as bass
import concourse.mybir as mybir
import concourse.tile as tile
from concourse import bacc
from concourse.bass_utils import run_bass_kernel_spmd
from concourse.masks import make_identity

# ---------------------------------------------------------------- constants
N_CORES = 8
B, C, H, W = 64, 240, 28, 28
NB = B // N_CORES           # images per core
KK = 5                      # depthwise kernel size
G = 60                      # groups of 4 channels
Cout = 240
R = 60                      # SE reduction dim
HW = H * W
EPS = 1e-5
NT = 392                    # pointwise free-dim tile (half an image)
XP = 121                    # arena partitions: block0 @0..57, block1 @64..121
NCB = 369                   # packed f32 const-blob columns

CFG = {
    "xchunks": (4, 8, 10, 12, 12, 14),  # group counts per x DMA chunk
    "tch": 4,               # groups per toeplitz rotation slot
    "trot": 4,              # toeplitz rotation depth
    "dwbufs": 4,            # DW psum pool depth
    "t2bufs": 4,            # T2 psum pool depth
    "copy_rr": "sv",        # T2 copy engine pattern (v=DVE, s=ACT)
    "d_rr": "sv",          # phase D epilogue engine pattern
    "stt_eng": "v",         # hardswish (a-3)*m engine
    "min_eng": "v",         # hardswish min engine
    "red_rr": "vs",         # SE reduce engines (v=DVE, s=ACT accum)
    "look": 18,             # input DMA lookahead (groups)
    "t2rate": 1,            # kc0 transpose quads interleaved per group
    "t2delay": 0,
    "pdbufs": 6,            # pointwise psum pool depth
}

F16 = mybir.dt.float16
F32 = mybir.dt.float32


# ---------------------------------------------------------------- builder
_BUILD_CACHE = {}


def build_nc(cfg_key=None):
    cfg = dict(CFG)
    if cfg_key is not None:
        cfg.update(cfg_key)
    key = tuple(sorted((k, str(v)) for k, v in cfg.items()))
    if key in _BUILD_CACHE:
        return _BUILD_CACHE[key]

    nc = bacc.Bacc("TRN2", target_bir_lowering=False, debug=False,
                   num_devices=N_CORES)

    xar_p = nc.declare_dram_parameter("xar", [XP, G, 2, 36, 4], F16,
                                      isOutput=False)
    tpar_p = nc.declare_dram_parameter("tpar", [XP, G, KK, 56], F16,
                                       isOutput=False)
    tpad_p = nc.declare_dram_parameter(
        "tpad", [XP, cfg["trot"] * cfg["tch"], 2, KK, 56], F16,
        isOutput=False)
    pwl_p = nc.declare_dram_parameter("pwl", [120, 2, 2, 120], F16,
                                      isOutput=False)
    cblob_p = nc.declare_dram_parameter("cblob", [120, NCB], F32,
                                        isOutput=False)
    y_p = nc.declare_dram_parameter("y", [NB, Cout, H, W], F16, isOutput=True)

    AL = mybir.AluOpType
    AF = mybir.ActivationFunctionType

    ENG = {"v": nc.vector, "s": nc.scalar, "g": nc.gpsimd}

    with tile.TileContext(nc) as tc:
        cst = tc.alloc_tile_pool(name="cst", bufs=1)
        pers = tc.alloc_tile_pool(name="pers", bufs=1)

        # ---- persistent arenas (chunked input DMAs for pipelined start)
        xchunks = list(cfg["xchunks"])
        assert sum(xchunks) == G
        xstarts = [sum(xchunks[:i]) for i in range(len(xchunks))]
        x_ch = [pers.tile([XP, n, 2, 36, 4], F16, name=f"xch{i}")
                for i, n in enumerate(xchunks)]
        xmap = {}
        for i, (s, n) in enumerate(zip(xstarts, xchunks)):
            for j in range(n):
                xmap[s + j] = (i, j)

        # toeplitz rotation slots: [121, TCH, blk, 5, 56]; the matmul rhs
        # view [:, g, :, dx, :] is the block-diagonal [121, 112].  Blk-major
        # keeps refill DMA runs at 560B (no small-transfer penalty).
        TCH, TROT = cfg["tch"], cfg["trot"]
        NTC = G // TCH
        t_sl = [pers.tile([XP, TCH, 2, KK, 56], F16, name=f"tsl{i}")
                for i in range(TROT)]

        def fill_toep(c):
            sl = t_sl[c % TROT]
            s = c * TCH
            if c < TROT:
                # first rotation: full-pad fill, zeros baked in DRAM
                nc.sync.dma_start(sl[:], tpad_p[:, s:s + TCH])
            else:
                # later refills only overwrite the payload block columns
                nc.sync.dma_start(sl[0:64, :, 0], tpar_p[0:64, s:s + TCH])
                nc.sync.dma_start(sl[57:121, :, 1],
                                  tpar_p[57:121, s:s + TCH])

        def issue_x(i):
            nc.sync.dma_start(x_ch[i][:],
                              xar_p[:, xstarts[i]:xstarts[i] + xchunks[i]])

        issue_x(0)
        fill_toep(0)
        issue_x(1)
        for i in range(1, TROT):
            fill_toep(i)

        # ---- constants in SBUF (single packed f32 blob + f16 pwl)
        pwl_sb = cst.tile([120, 2, 2, 120], F16)       # [K=c, kc, mo, M=o]
        nc.sync.dma_start(pwl_sb[:], pwl_p[:])
        cblob = cst.tile([120, NCB], F32)
        nc.sync.dma_start(cblob[:], cblob_p[:])
        se1l_sb = cblob[:, 0:120].rearrange("p (kc r) -> p kc r", kc=2)
        se1b_sb = cblob[0:R, 120:121]
        se2l_sb = cblob[0:R, 121:361].rearrange("p (mo o) -> p mo o", mo=2)
        se2b3_sb = cblob[:, 361:363]
        bn2s_sb = cblob[:, 363:365]
        bn2sb_sb = cblob[:, 365:367]
        bn2t_sb = cblob[:, 367:369]

        ident = cst.tile([128, 128], F16)
        make_identity(nc, ident[:])

        # remaining input DMAs are emitted inside the group loop, ordered
        # by the group that first needs them (see dma_sched)

        # D: depthwise+HS output, [(4b,w), half, g, c4, h]
        D = pers.tile([112, 2, G, 4, H], F16, name="dact")
        # R: channel-major pointwise rhs per kc chunk: [c, half, b4, h, w]
        Rt = [pers.tile([120, 2, 4, H, W], F16, name=f"rt{kc}")
              for kc in range(2)]
        s_sb = [pers.tile([120, NB], F32, name=f"s{kc}") for kc in range(2)]
        g_sb = [pers.tile([120, NB], F32, name=f"gate{mo}") for mo in range(2)]
        sc2 = [pers.tile([120, NB], F32, name=f"sc2{mo}") for mo in range(2)]
        bi2 = [pers.tile([120, NB], F32, name=f"bi2{mo}") for mo in range(2)]

        # input-DMA emission schedule: group -> [(fn, arg), ...], ordered by
        # first-need time with a few groups of lookahead
        LOOK = cfg.get("look", 12)
        dma_sched = {}
        for i in range(2, len(xchunks)):
            dma_sched.setdefault(max(0, xstarts[i] - LOOK), []).append(
                (issue_x, i))
        for c in range(TROT, NTC):
            dma_sched.setdefault((c - TROT) * TCH + TCH - 1, []).append(
                (fill_toep, c))

        # ================= Phase A: depthwise + BN1 + hardswish
        copy_rr = cfg["copy_rr"]
        cp_i = 0
        if True:
            pa = tc.alloc_tile_pool(name="pa", bufs=4)
            dwps = tc.alloc_tile_pool(name="dwps", bufs=cfg["dwbufs"],
                                      space="PSUM")
            t2ps = tc.alloc_tile_pool(name="t2ps", bufs=cfg["t2bufs"],
                                      space="PSUM", side="right")

            stt_eng = ENG[cfg["stt_eng"]]

            def do_group(g):
                ci, co = xmap[g]
                xg = x_ch[ci]
                tg, to = t_sl[(g // TCH) % TROT], g % TCH
                ps = dwps.tile([128, 2, 112], F32, tag="dw")
                n = 0
                for half in (0, 1):
                    for dx in range(KK):
                        # lhsT free (32w, 4b) strides (4,1) merges to (128,1)
                        nc.tensor.matmul(
                            ps[:, half, :],
                            xg[:, co, half, dx:dx + 32, :],
                            tg[:, to, :, dx, :],
                            start=(n == 0), stop=(n == 9))
                        n += 1
                a = pa.tile([112, 2, 112], F16, tag="a")
                nc.scalar.activation(a[:], ps[0:112], AF.Relu)
                m = pa.tile([112, 2, 112], F16, tag="m")
                ENG[cfg.get("min_eng", "v")].tensor_scalar(
                    m[:], a[:], 1.0 / 6.0, 1.0, AL.mult, AL.min)
                dst = D[:, :, g, :, :].rearrange("p half c h -> p half (c h)")
                stt_eng.scalar_tensor_tensor(dst, a[:], 3.0, m[:],
                                             AL.subtract, AL.mult)
                # emit upcoming input DMAs in the order they will be needed
                for fn, arg in dma_sched.get(g, ()):
                    fn(arg)

            red_rr = cfg["red_rr"]
            trash = pers.tile([120, HW], F16, name="trash")
            rd_i = 0

            def t2_quad(kc, half, h0):
                nonlocal cp_i
                tp = t2ps.tile([120, 4, 112], F16, tag="t2")
                for hi in range(4):
                    nc.tensor.transpose(
                        tp[:, hi, :],
                        D[:, half, 30 * kc:30 * kc + 30, :, h0 + hi],
                        ident[:112, :112])
                e = copy_rr[cp_i % len(copy_rr)]
                cp_i += 1
                cdst = Rt[kc][:, half, :, h0:h0 + 4, :]
                csrc = tp[:].rearrange("p hh (w b) -> p b hh w", b=4)
                if e == "s":
                    nc.scalar.copy(cdst, csrc)
                else:
                    ENG[e].tensor_copy(cdst, csrc)

            def t2_reduce(kc, half):
                nonlocal rd_i
                # SE partial reduces for this (kc, half) as soon as ready
                for b4 in range(4):
                    scol = s_sb[kc][:, 4 * half + b4:4 * half + b4 + 1]
                    rsrc = Rt[kc][:, half, b4].rearrange("p h w -> p (h w)")
                    e = red_rr[rd_i % len(red_rr)]
                    rd_i += 1
                    if e == "s":
                        nc.scalar.activation(trash[:], rsrc, AF.Identity,
                                             accum_out=scol)
                    else:
                        nc.vector.tensor_reduce(
                            scol, rsrc, mybir.AxisListType.X, AL.add)

            def do_t2(kc, half):
                for h0 in range(0, H, 4):
                    t2_quad(kc, half, h0)
                t2_reduce(kc, half)

            for g in range(30):
                do_group(g)
            # interleave the kc0 transpose quads with groups 30..: one quad
            # per group keeps the DW stream and input DMAs flowing
            t2q = [(0, half, h0) for half in (0, 1) for h0 in range(0, H, 4)]
            T2D = cfg.get("t2delay", 0)
            T2R = cfg.get("t2rate", 1)
            for g in range(30, 60):
                do_group(g)
                for k in range(T2R):
                    qi = (g - 30 - T2D) * T2R + k
                    if 0 <= qi < len(t2q):
                        t2_quad(*t2q[qi])
                        if qi == 6:
                            t2_reduce(0, 0)
                        elif qi == 13:
                            t2_reduce(0, 1)
            dwps.release()
            pa.release()

        # ================= Phase C + D, pipelined per image-half
        d_rr = cfg["d_rr"]
        d_i = 0
        rflat = [Rt[kc][:].rearrange("p half b h w -> p (half b h w)")
                 for kc in range(2)]
        # output staging: [120, half, b4, h, w] per mo; 1 DMA per (mo, half)
        ystage = [pers.tile([120, 2, 4, H, W], F16, name=f"ys{mo}")
                  for mo in range(2)]
        ysflat = [ystage[mo][:].rearrange("p half b h w -> p (half b h w)")
                  for mo in range(2)]
        seps = tc.alloc_tile_pool(name="seps", bufs=1, space="PSUM")
        pdps = None

        def se_chain(half):
            """gate + fused BN2 scale/bias for images 4*half..4*half+4"""
            hs = slice(4 * half, 4 * half + 4)
            ps1 = seps.tile([R, 4], F32, tag="se1")
            for kc in range(2):
                nc.tensor.matmul(ps1[:], se1l_sb[:, kc, :], s_sb[kc][:, hs],
                                 start=(kc == 0), stop=(kc == 1))
            h1 = pers.tile([R, 4], F32, name=f"h1_{half}")
            nc.scalar.activation(h1[:], ps1[:], AF.Relu, bias=se1b_sb)
            for mo in range(2):
                ps2 = seps.tile([120, 4], F32, tag="se2")
                nc.tensor.matmul(ps2[:], se2l_sb[:, mo, :], h1[:],
                                 start=True, stop=True)
                a2 = pers.tile([120, 4], F32, name=f"a2_{half}_{mo}")
                nc.scalar.activation(a2[:], ps2[:], AF.Relu,
                                     bias=se2b3_sb[:, mo:mo + 1])
                m2 = pers.tile([120, 4], F32, name=f"m2_{half}_{mo}")
                nc.vector.tensor_scalar(m2[:], a2[:], 1.0 / 6.0, 1.0,
                                        AL.mult, AL.min)
                nc.vector.scalar_tensor_tensor(g_sb[mo][:, hs], a2[:], 3.0,
                                               m2[:], AL.subtract, AL.mult)
                # scale2 = s2*g ; bias2 = (s2*pw_b)*g + t2  (per image col)
                nc.vector.tensor_tensor(
                    sc2[mo][:, hs], g_sb[mo][:, hs],
                    bn2s_sb[:, mo:mo + 1].to_broadcast((120, 4)), AL.mult)
                tmpb = pers.tile([120, 4], F32, name=f"tb_{half}_{mo}")
                nc.gpsimd.tensor_tensor(
                    tmpb[:], g_sb[mo][:, hs],
                    bn2sb_sb[:, mo:mo + 1].to_broadcast((120, 4)), AL.mult)
                nc.gpsimd.tensor_tensor(
                    bi2[mo][:, hs], tmpb[:],
                    bn2t_sb[:, mo:mo + 1].to_broadcast((120, 4)), AL.add)

        def do_d(half):
            nonlocal d_i
            for mo in range(2):
                for b in range(4 * half, 4 * half + 4):
                    for nt in range(2):
                        off = b * HW + nt * NT
                        ps = pdps.tile([120, NT], F32, tag="pw")
                        for kc in range(2):
                            nc.tensor.matmul(ps[:], pwl_sb[:, kc, mo, :],
                                             rflat[kc][:, off:off + NT],
                                             start=(kc == 0), stop=(kc == 1))
                        o = ysflat[mo][:, off:off + NT]
                        e = d_rr[d_i % len(d_rr)]
                        d_i += 1
                        if e == "s":
                            nc.scalar.activation(o, ps[:], AF.Identity,
                                                 bias=bi2[mo][:, b:b + 1],
                                                 scale=sc2[mo][:, b:b + 1])
                        else:
                            ENG[e].tensor_scalar(o, ps[:], sc2[mo][:, b:b + 1],
                                                 bi2[mo][:, b:b + 1],
                                                 AL.mult, AL.add)
                nyd = 4 if half == 1 else 2
                for bp in range(nyd):
                    w0 = 4 // nyd
                    b0 = 4 * half + w0 * bp
                    y_ap = y_p[b0:b0 + w0,
                               mo * 120:(mo + 1) * 120].rearrange(
                        "b c h w -> c b (h w)")
                    nc.sync.dma_start(
                        y_ap,
                        ystage[mo][:, half, w0 * bp:w0 * bp + w0].rearrange(
                            "p b h w -> p b (h w)"))

        if cfg.get("d_interleave"):
            pdps = tc.alloc_tile_pool(name="pdps", bufs=cfg["pdbufs"],
                                      space="PSUM")
            do_t2(1, 0)
            se_chain(0)
            do_d(0)
            do_t2(1, 1)
            se_chain(1)
            do_d(1)
            t2ps.release()
        else:
            do_t2(1, 0)
            do_t2(1, 1)
            t2ps.release()
            pdps = tc.alloc_tile_pool(name="pdps", bufs=cfg["pdbufs"],
                                      space="PSUM")
            se_chain(0)
            se_chain(1)
            do_d(0)
            do_d(1)

        pdps.release()
        seps.release()
        pers.release()
        cst.release()

    nc.compile()
    _BUILD_CACHE[key] = nc
    return nc


# ---------------------------------------------------------------- host prep
def prep_inputs(inputs, cfg_key=None):
    f32, f16 = np.float32, np.float16

    x = np.asarray(inputs["x"], f32)
    dw_w = np.asarray(inputs["dw_w"], f32)      # [C,1,5,5]
    dw_b = np.asarray(inputs["dw_b"], f32)
    bn1_g = np.asarray(inputs["bn1_g"], f32)
    bn1_b = np.asarray(inputs["bn1_b"], f32)
    bn1_m = np.asarray(inputs["bn1_m"], f32)
    bn1_v = np.asarray(inputs["bn1_v"], f32)
    pw_w = np.asarray(inputs["pw_w"], f32)      # [Cout, C]
    pw_b = np.asarray(inputs["pw_b"], f32)
    se_w1 = np.asarray(inputs["se_w1"], f32)    # [R, C]
    se_b1 = np.asarray(inputs["se_b1"], f32)
    se_w2 = np.asarray(inputs["se_w2"], f32)    # [Cout, R]
    se_b2 = np.asarray(inputs["se_b2"], f32)
    bn2_g = np.asarray(inputs["bn2_g"], f32)
    bn2_b = np.asarray(inputs["bn2_b"], f32)
    bn2_m = np.asarray(inputs["bn2_m"], f32)
    bn2_v = np.asarray(inputs["bn2_v"], f32)

    s1 = bn1_g / np.sqrt(bn1_v + EPS)
    t1 = s1 * (dw_b - bn1_m) + bn1_b

    # Compact Toeplitz [XP, G, KK, 56]: block kb rows base_k + 28*c_in + h_in
    # hold s1[ch]*w[ch, h_in-h_out+2, dx] at col 28*c_in + h_out; row
    # base_k+56 holds t1+3 (dx=0 only).  The device expands this to the
    # block-diagonal [121, 112] rhs via two column-offset DMAs per slot.
    hin = np.arange(H)[:, None]
    hout = np.arange(H)[None, :]
    Dh = hin - hout
    mask = np.abs(Dh) <= 2
    dyi = np.clip(Dh + 2, 0, 4)
    k = dw_w[:, 0] * s1[:, None, None]                        # [C, 5, 5]
    band = np.where(mask[None, :, :, None], k[:, dyi, :], 0.0)  # [C,hin,hout,dx]
    tpar = np.zeros((XP, G, KK, 56), f32)
    for kb in range(2):
        base = 64 * kb
        for ci in range(2):
            ch = np.arange(G) * 4 + 2 * kb + ci               # [G]
            col = 28 * ci
            tpar[base + 28 * ci:base + 28 * ci + 28, :, :,
                 col:col + 28] = \
                band[ch].transpose(1, 0, 3, 2)                # [hin, G, dx, hout]
            tpar[base + 56, :, 0, col:col + 28] = \
                (t1[ch] + 3.0)[:, None]
    tpar = tpar.astype(f16)

    # pointwise weights [K=c(120), kc, mo, M=o(120)]
    pwT = pw_w.T                                              # [C, Cout]
    pwl = np.zeros((120, 2, 2, 120), f32)
    for kc in range(2):
        for mo in range(2):
            pwl[:, kc, mo, :] = pwT[kc * 120:(kc + 1) * 120,
                                    mo * 120:(mo + 1) * 120]
    pwl = pwl.astype(f16)

    s2 = bn2_g / np.sqrt(bn2_v + EPS)
    cblob = np.zeros((120, NCB), f32)
    # se1l [120, (kc, r)] = w1T[kc*120+p, r] / HW
    cblob[:, 0:120] = (se_w1.T / HW).reshape(2, 120, R).transpose(
        1, 0, 2).reshape(120, 120)
    cblob[:R, 120] = se_b1
    cblob[:R, 121:361] = se_w2.T.reshape(R, 240)
    cblob[:, 361:363] = (se_b2 + 3.0).reshape(2, 120).T
    cblob[:, 363:365] = s2.reshape(2, 120).T
    cblob[:, 365:367] = (s2 * pw_b).reshape(2, 120).T
    cblob[:, 367:369] = (bn2_b - bn2_m * s2).reshape(2, 120).T

    npad = CFG["trot"] * CFG["tch"]
    tpad = np.zeros((XP, npad, 2, KK, 56), f16)
    tpad[0:64, :, 0] = tpar[0:64, 0:npad].transpose(0, 1, 2, 3)
    tpad[57:121, :, 1] = tpar[57:121, 0:npad]
    shared = {
        "tpar": tpar, "pwl": pwl, "cblob": cblob, "tpad": tpad,
    }

    # x arena [XP, G, half, 36w, 4b]: rows base_k + 28*c_loc + h hold
    # x[4*half+b4, ch, h, j-2] (zero padded in w); row base_k+56 = 1.0
    x16 = x.astype(f16)
    in_maps = []
    for core in range(N_CORES):
        xc = x16[core * NB:(core + 1) * NB]                   # [NB, C, H, W]
        xh = xc.reshape(2, 4, C, H, W)                        # [half, b4, ...]
        xar = np.zeros((XP, G, 2, 36, 4), f16)
        for kb in range(2):
            base = 64 * kb
            for ci in range(2):
                ch = np.arange(G) * 4 + 2 * kb + ci
                # [half, b4, G, H, W] -> [H, G, half, W, b4]
                xar[base + 28 * ci:base + 28 * ci + 28, :, :, 2:2 + W, :] = \
                    xh[:, :, ch].transpose(3, 2, 0, 4, 1)
            xar[base + 56] = 1.0
        m = dict(shared)
        m["xar"] = xar
        in_maps.append(m)
    return in_maps


def kernel(**inputs):
    nc = build_nc()
    in_maps = prep_inputs(inputs)
    res = run_bass_kernel_spmd(nc, in_maps, list(range(N_CORES)))
    out = np.concatenate(
        [np.asarray(res.results[i]["y"]) for i in range(N_CORES)], axis=0)
    return out.astype(np.float32)


# revision 68
# speedup vs baseline: 1.0173x; 1.0173x over previous
"""Trainium2 Bass kernel for nn_DepthWiseSepConv (depthwise 5x5 + BN+hardswish
+ pointwise 1x1 + squeeze-excite gating + BN), data-parallel over batch on
8 NeuronCores.

Self-contained: hardcodes all shapes from the problem spec.

Per-core layout (B_loc = 8 images per core), all matmul operands fp16:

  - Depthwise conv, operand-swapped Toeplitz form: stationary lhsT = x
    [(2ch x 28h_in) + ones-row, (4b x 28w_out)], moving rhs = per-group
    block-diagonal Toeplitz [(2ch x 28h_in) + bias-row, (2ch x 28h_out)].
    Output lands pixel-major [(32w, 4b), (c4, h)] so only ONE transpose
    stage is needed (to channel-major) before the pointwise conv.  The x
    arena is packed (w-major, b-inner) so the stationary operand's free
    dims merge to a single walrus-legal dimension.  BN1 scale is folded
    into the Toeplitz weights; BN1 bias (+3 for the hardswish trick) rides
    an extra ones-row of x (dx=0 slice only).  5 dx shifts accumulate in
    PSUM.  Channel blocks sit at partition bases 0 and 64 (tile_position
    rule); the first rotation of Toeplitz slots is filled full-pad (zeros
    baked in DRAM), later refills overwrite only the payload columns.
  - Hardswish epilogue: ACT relu (PSUM->f16), DVE min, DVE (a-3)*m -> D.
  - One transpose stage (T2): [(4b,w), (g,c)@h] -> channel-major R tiles.
  - SE: per-image DVE reduces on R, two small matmuls, hardswish; gate and
    BN2 are folded into per-(image,half) ACT scale/bias vectors.
  - Pointwise: [120c x 120o] fp16 matmuls, N=392, single fused epilogue op.
"""

import sys

sys.path.insert(0, "/opt/trn_rl_repo")

import numpy as np

import concourse.bass as bass
import concourse.mybir as mybir
import concourse.tile as tile
from concourse import bacc
from concourse.bass_utils import run_bass_kernel_spmd
from concourse.masks import make_identity

# ---------------------------------------------------------------- constants
N_CORES = 8
B, C, H, W = 64, 240, 28, 28
NB = B // N_CORES           # images per core
KK = 5                      # depthwise kernel size
G = 60                      # groups of 4 channels
Cout = 240
R = 60                      # SE reduction dim
HW = H * W
EPS = 1e-5
NT = 392                    # pointwise free-dim tile (half an image)
XP = 121                    # arena partitions: block0 @0..57, block1 @64..121
NCB = 369                   # packed f32 const-blob columns

CFG = {
    "xchunks": (4, 8, 10, 12, 14, 12),  # group counts per x DMA chunk
    "tch": 4,               # groups per toeplitz rotation slot
    "trot": 4,              # toeplitz rotation depth
    "dwbufs": 4,            # DW psum pool depth
    "t2bufs": 4,            # T2 psum pool depth
    "copy_rr": "sv",        # T2 copy engine pattern (v=DVE, s=ACT)
    "d_rr": "sv",          # phase D epilogue engine pattern
    "stt_eng": "v",         # hardswish (a-3)*m engine
    "min_eng": "v",         # hardswish min engine
    "red_rr": "vs",         # SE reduce engines (v=DVE, s=ACT accum)
    "look": 14,             # input DMA lookahead (groups)
    "t2rate": 1,            # kc0 transpose quads interleaved per group
    "t2delay": 0,
    "pdbufs": 6,            # pointwise psum pool depth
}

F16 = mybir.dt.float16
F32 = mybir.dt.float32


# ---------------------------------------------------------------- builder
_BUILD_CACHE = {}


def build_nc(cfg_key=None):
    cfg = dict(CFG)
    if cfg_key is not None:
        cfg.update(cfg_key)
    key = tuple(sorted((k, str(v)) for k, v in cfg.items()))
    if key in _BUILD_CACHE:
        return _BUILD_CACHE[key]

    nc = bacc.Bacc("TRN2", target_bir_lowering=False, debug=False,
                   num_devices=N_CORES)

    xar_p = nc.declare_dram_parameter("xar", [XP, G, 2, 36, 4], F16,
                                      isOutput=False)
    tpar_p = nc.declare_dram_parameter("tpar", [XP, G, KK, 56], F16,
                                       isOutput=False)
    tpad_p = nc.declare_dram_parameter(
        "tpad", [XP, cfg["trot"] * cfg["tch"], 2, KK, 56], F16,
        isOutput=False)
    pwl_p = nc.declare_dram_parameter("pwl", [120, 2, 2, 120], F16,
                                      isOutput=False)
    cblob_p = nc.declare_dram_parameter("cblob", [120, NCB], F32,
                                        isOutput=False)
    y_p = nc.declare_dram_parameter("y", [NB, Cout, H, W], F16, isOutput=True)

    AL = mybir.AluOpType
    AF = mybir.ActivationFunctionType

    ENG = {"v": nc.vector, "s": nc.scalar, "g": nc.gpsimd}

    with tile.TileContext(nc) as tc:
        cst = tc.alloc_tile_pool(name="cst", bufs=1)
        pers = tc.alloc_tile_pool(name="pers", bufs=1)

        # ---- persistent arenas (chunked input DMAs for pipelined start)
        xchunks = list(cfg["xchunks"])
        assert sum(xchunks) == G
        xstarts = [sum(xchunks[:i]) for i in range(len(xchunks))]
        x_ch = [pers.tile([XP, n, 2, 36, 4], F16, name=f"xch{i}")
                for i, n in enumerate(xchunks)]
        xmap = {}
        for i, (s, n) in enumerate(zip(xstarts, xchunks)):
            for j in range(n):
                xmap[s + j] = (i, j)

        # toeplitz rotation slots: [121, TCH, blk, 5, 56]; the matmul rhs
        # view [:, g, :, dx, :] is the block-diagonal [121, 112].  Blk-major
        # keeps refill DMA runs at 560B (no small-transfer penalty).
        TCH, TROT = cfg["tch"], cfg["trot"]
        NTC = G // TCH
        t_sl = [pers.tile([XP, TCH, 2, KK, 56], F16, name=f"tsl{i}")
                for i in range(TROT)]

        def fill_toep(c):
            sl = t_sl[c % TROT]
            s = c * TCH
            if c < TROT:
                # first rotation: full-pad fill, zeros baked in DRAM
                nc.sync.dma_start(sl[:], tpad_p[:, s:s + TCH])
            else:
                # later refills only overwrite the payload block columns
                nc.sync.dma_start(sl[0:64, :, 0], tpar_p[0:64, s:s + TCH])
                nc.sync.dma_start(sl[57:121, :, 1],
                                  tpar_p[57:121, s:s + TCH])

        def issue_x(i):
            nc.sync.dma_start(x_ch[i][:],
                              xar_p[:, xstarts[i]:xstarts[i] + xchunks[i]])

        issue_x(0)
        fill_toep(0)
        issue_x(1)
        for i in range(1, TROT):
            fill_toep(i)

        # ---- constants in SBUF (single packed f32 blob + f16 pwl)
        pwl_sb = cst.tile([120, 2, 2, 120], F16)       # [K=c, kc, mo, M=o]
        nc.sync.dma_start(pwl_sb[:], pwl_p[:])
        cblob = cst.tile([120, NCB], F32)
        nc.sync.dma_start(cblob[:], cblob_p[:])
        se1l_sb = cblob[:, 0:120].rearrange("p (kc r) -> p kc r", kc=2)
        se1b_sb = cblob[0:R, 120:121]
        se2l_sb = cblob[0:R, 121:361].rearrange("p (mo o) -> p mo o", mo=2)
        se2b3_sb = cblob[:, 361:363]
        bn2s_sb = cblob[:, 363:365]
        bn2sb_sb = cblob[:, 365:367]
        bn2t_sb = cblob[:, 367:369]

        ident = cst.tile([128, 128], F16)
        make_identity(nc, ident[:])

        # remaining input DMAs are emitted inside the group loop, ordered
        # by the group that first needs them (see dma_sched)

        # D: depthwise+HS output, [(4b,w), half, g, c4, h]
        D = pers.tile([112, 2, G, 4, H], F16, name="dact")
        # R: channel-major pointwise rhs per kc chunk: [c, half, b4, h, w]
        Rt = [pers.tile([120, 2, 4, H, W], F16, name=f"rt{kc}")
              for kc in range(2)]
        s_sb = [pers.tile([120, NB], F32, name=f"s{kc}") for kc in range(2)]
        g_sb = [pers.tile([120, NB], F32, name=f"gate{mo}") for mo in range(2)]
        sc2 = [pers.tile([120, NB], F32, name=f"sc2{mo}") for mo in range(2)]
        bi2 = [pers.tile([120, NB], F32, name=f"bi2{mo}") for mo in range(2)]

        # input-DMA emission schedule: group -> [(fn, arg), ...], ordered by
        # first-need time with a few groups of lookahead
        LOOK = cfg.get("look", 12)
        dma_sched = {}
        for i in range(2, len(xchunks)):
            dma_sched.setdefault(max(0, xstarts[i] - LOOK), []).append(
                (issue_x, i))
        for c in range(TROT, NTC):
            dma_sched.setdefault((c - TROT) * TCH + TCH - 1, []).append(
                (fill_toep, c))

        # ================= Phase A: depthwise + BN1 + hardswish
        copy_rr = cfg["copy_rr"]
        cp_i = 0
        if True:
            pa = tc.alloc_tile_pool(name="pa", bufs=4)
            dwps = tc.alloc_tile_pool(name="dwps", bufs=cfg["dwbufs"],
                                      space="PSUM")
            t2ps = tc.alloc_tile_pool(name="t2ps", bufs=cfg["t2bufs"],
                                      space="PSUM", side="right")

            stt_eng = ENG[cfg["stt_eng"]]

            def do_group(g):
                ci, co = xmap[g]
                xg = x_ch[ci]
                tg, to = t_sl[(g // TCH) % TROT], g % TCH
                ps = dwps.tile([128, 2, 112], F32, tag="dw")
                n = 0
                for half in (0, 1):
                    for dx in range(KK):
                        # lhsT free (32w, 4b) strides (4,1) merges to (128,1)
                        nc.tensor.matmul(
                            ps[:, half, :],
                            xg[:, co, half, dx:dx + 32, :],
                            tg[:, to, :, dx, :],
                            start=(n == 0), stop=(n == 9))
                        n += 1
                a = pa.tile([112, 2, 112], F16, tag="a")
                nc.scalar.activation(a[:], ps[0:112], AF.Relu)
                m = pa.tile([112, 2, 112], F16, tag="m")
                ENG[cfg.get("min_eng", "v")].tensor_scalar(
                    m[:], a[:], 1.0 / 6.0, 1.0, AL.mult, AL.min)
                dst = D[:, :, g, :, :].rearrange("p half c h -> p half (c h)")
                stt_eng.scalar_tensor_tensor(dst, a[:], 3.0, m[:],
                                             AL.subtract, AL.mult)
                # emit upcoming input DMAs in the order they will be needed
                for fn, arg in dma_sched.get(g, ()):
                    fn(arg)

            red_rr = cfg["red_rr"]
            trash = pers.tile([120, HW], F16, name="trash")
            rd_i = 0

            def t2_quad(kc, half, h0):
                nonlocal cp_i
                tp = t2ps.tile([120, 4, 112], F16, tag="t2")
                for hi in range(4):
                    nc.tensor.transpose(
                        tp[:, hi, :],
                        D[:, half, 30 * kc:30 * kc + 30, :, h0 + hi],
                        ident[:112, :112])
                e = copy_rr[cp_i % len(copy_rr)]
                cp_i += 1
                cdst = Rt[kc][:, half, :, h0:h0 + 4, :]
                csrc = tp[:].rearrange("p hh (w b) -> p b hh w", b=4)
                if e == "s":
                    nc.scalar.copy(cdst, csrc)
                else:
                    ENG[e].tensor_copy(cdst, csrc)

            def t2_reduce(kc, half):
                nonlocal rd_i
                # SE partial reduces for this (kc, half) as soon as ready
                for b4 in range(4):
                    scol = s_sb[kc][:, 4 * half + b4:4 * half + b4 + 1]
                    rsrc = Rt[kc][:, half, b4].rearrange("p h w -> p (h w)")
                    e = red_rr[rd_i % len(red_rr)]
                    rd_i += 1
                    if e == "s":
                        nc.scalar.activation(trash[:], rsrc, AF.Identity,
                                             accum_out=scol)
                    else:
                        nc.vector.tensor_reduce(
                            scol, rsrc, mybir.AxisListType.X, AL.add)

            def do_t2(kc, half):
                for h0 in range(0, H, 4):
                    t2_quad(kc, half, h0)
                t2_reduce(kc, half)

            for g in range(30):
                do_group(g)
            # interleave the kc0 transpose quads with groups 30..: one quad
            # per group keeps the DW stream and input DMAs flowing
            t2q = [(0, half, h0) for half in (0, 1) for h0 in range(0, H, 4)]
            T2D = cfg.get("t2delay", 0)
            T2R = cfg.get("t2rate", 1)
            for g in range(30, 60):
                do_group(g)
                for k in range(T2R):
                    qi = (g - 30 - T2D) * T2R + k
                    if 0 <= qi < len(t2q):
                        t2_quad(*t2q[qi])
                        if qi == 6:
                            t2_reduce(0, 0)
                        elif qi == 13:
                            t2_reduce(0, 1)
            dwps.release()
            pa.release()

        # ================= Phase C + D, pipelined per image-half
        d_rr = cfg["d_rr"]
        d_i = 0
        rflat = [Rt[kc][:].rearrange("p half b h w -> p (half b h w)")
                 for kc in range(2)]
        # output staging: [120, half, b4, h, w] per mo; 1 DMA per (mo, half)
        ystage = [pers.tile([120, 2, 4, H, W], F16, name=f"ys{mo}")
                  for mo in range(2)]
        ysflat = [ystage[mo][:].rearrange("p half b h w -> p (half b h w)")
                  for mo in range(2)]
        seps = tc.alloc_tile_pool(name="seps", bufs=1, space="PSUM")
        pdps = None

        def se_chain(half):
            """gate + fused BN2 scale/bias for images 4*half..4*half+4"""
            hs = slice(4 * half, 4 * half + 4)
            ps1 = seps.tile([R, 4], F32, tag="se1")
            for kc in range(2):
                nc.tensor.matmul(ps1[:], se1l_sb[:, kc, :], s_sb[kc][:, hs],
                                 start=(kc == 0), stop=(kc == 1))
            h1 = pers.tile([R, 4], F32, name=f"h1_{half}")
            nc.scalar.activation(h1[:], ps1[:], AF.Relu, bias=se1b_sb)
            for mo in range(2):
                ps2 = seps.tile([120, 4], F32, tag="se2")
                nc.tensor.matmul(ps2[:], se2l_sb[:, mo, :], h1[:],
                                 start=True, stop=True)
                a2 = pers.tile([120, 4], F32, name=f"a2_{half}_{mo}")
                nc.scalar.activation(a2[:], ps2[:], AF.Relu,
                                     bias=se2b3_sb[:, mo:mo + 1])
                m2 = pers.tile([120, 4], F32, name=f"m2_{half}_{mo}")
                nc.vector.tensor_scalar(m2[:], a2[:], 1.0 / 6.0, 1.0,
                                        AL.mult, AL.min)
                nc.vector.scalar_tensor_tensor(g_sb[mo][:, hs], a2[:], 3.0,
                                               m2[:], AL.subtract, AL.mult)
                # scale2 = s2*g ; bias2 = (s2*pw_b)*g + t2  (per image col)
                nc.vector.tensor_tensor(
                    sc2[mo][:, hs], g_sb[mo][:, hs],
                    bn2s_sb[:, mo:mo + 1].to_broadcast((120, 4)), AL.mult)
                tmpb = pers.tile([120, 4], F32, name=f"tb_{half}_{mo}")
                nc.gpsimd.tensor_tensor(
                    tmpb[:], g_sb[mo][:, hs],
                    bn2sb_sb[:, mo:mo + 1].to_broadcast((120, 4)), AL.mult)
                nc.gpsimd.tensor_tensor(
                    bi2[mo][:, hs], tmpb[:],
                    bn2t_sb[:, mo:mo + 1].to_broadcast((120, 4)), AL.add)

        def do_d(half):
            nonlocal d_i
            for mo in range(2):
                for b in range(4 * half, 4 * half + 4):
                    for nt in range(2):
                        off = b * HW + nt * NT
                        ps = pdps.tile([120, NT], F32, tag="pw")
                        for kc in range(2):
                            nc.tensor.matmul(ps[:], pwl_sb[:, kc, mo, :],
                                             rflat[kc][:, off:off + NT],
                                             start=(kc == 0), stop=(kc == 1))
                        o = ysflat[mo][:, off:off + NT]
                        e = d_rr[d_i % len(d_rr)]
                        d_i += 1
                        if e == "s":
                            nc.scalar.activation(o, ps[:], AF.Identity,
                                                 bias=bi2[mo][:, b:b + 1],
                                                 scale=sc2[mo][:, b:b + 1])
                        else:
                            ENG[e].tensor_scalar(o, ps[:], sc2[mo][:, b:b + 1],
                                                 bi2[mo][:, b:b + 1],
                                                 AL.mult, AL.add)
                nyd = 4 if half == 1 else 2
                for bp in range(nyd):
                    w0 = 4 // nyd
                    b0 = 4 * half + w0 * bp
                    y_ap = y_p[b0:b0 + w0,
                               mo * 120:(mo + 1) * 120].rearrange(
                        "b c h w -> c b (h w)")
                    nc.sync.dma_start(
                        y_ap,
                        ystage[mo][:, half, w0 * bp:w0 * bp + w0].rearrange(
                            "p b h w -> p b (h w)"))

        if cfg.get("d_interleave"):
            pdps = tc.alloc_tile_pool(name="pdps", bufs=cfg["pdbufs"],
                                      space="PSUM")
            do_t2(1, 0)
            se_chain(0)
            do_d(0)
            do_t2(1, 1)
            se_chain(1)
            do_d(1)
            t2ps.release()
        else:
            do_t2(1, 0)
            do_t2(1, 1)
            t2ps.release()
            pdps = tc.alloc_tile_pool(name="pdps", bufs=cfg["pdbufs"],
                                      space="PSUM")
            se_chain(0)
            se_chain(1)
            do_d(0)
            do_d(1)

        pdps.release()
        seps.release()
        pers.release()
        cst.release()

    nc.compile()
    _BUILD_CACHE[key] = nc
    return nc


# ---------------------------------------------------------------- host prep
def prep_inputs(inputs, cfg_key=None):
    f32, f16 = np.float32, np.float16

    x = np.asarray(inputs["x"], f32)
    dw_w = np.asarray(inputs["dw_w"], f32)      # [C,1,5,5]
    dw_b = np.asarray(inputs["dw_b"], f32)
    bn1_g = np.asarray(inputs["bn1_g"], f32)
    bn1_b = np.asarray(inputs["bn1_b"], f32)
    bn1_m = np.asarray(inputs["bn1_m"], f32)
    bn1_v = np.asarray(inputs["bn1_v"], f32)
    pw_w = np.asarray(inputs["pw_w"], f32)      # [Cout, C]
    pw_b = np.asarray(inputs["pw_b"], f32)
    se_w1 = np.asarray(inputs["se_w1"], f32)    # [R, C]
    se_b1 = np.asarray(inputs["se_b1"], f32)
    se_w2 = np.asarray(inputs["se_w2"], f32)    # [Cout, R]
    se_b2 = np.asarray(inputs["se_b2"], f32)
    bn2_g = np.asarray(inputs["bn2_g"], f32)
    bn2_b = np.asarray(inputs["bn2_b"], f32)
    bn2_m = np.asarray(inputs["bn2_m"], f32)
    bn2_v = np.asarray(inputs["bn2_v"], f32)

    s1 = bn1_g / np.sqrt(bn1_v + EPS)
    t1 = s1 * (dw_b - bn1_m) + bn1_b

    # Compact Toeplitz [XP, G, KK, 56]: block kb rows base_k + 28*c_in + h_in
    # hold s1[ch]*w[ch, h_in-h_out+2, dx] at col 28*c_in + h_out; row
    # base_k+56 holds t1+3 (dx=0 only).  The device expands this to the
    # block-diagonal [121, 112] rhs via two column-offset DMAs per slot.
    hin = np.arange(H)[:, None]
    hout = np.arange(H)[None, :]
    Dh = hin - hout
    mask = np.abs(Dh) <= 2
    dyi = np.clip(Dh + 2, 0, 4)
    k = dw_w[:, 0] * s1[:, None, None]                        # [C, 5, 5]
    band = np.where(mask[None, :, :, None], k[:, dyi, :], 0.0)  # [C,hin,hout,dx]
    tpar = np.zeros((XP, G, KK, 56), f32)
    for kb in range(2):
        base = 64 * kb
        for ci in range(2):
            ch = np.arange(G) * 4 + 2 * kb + ci               # [G]
            col = 28 * ci
            tpar[base + 28 * ci:base + 28 * ci + 28, :, :,
                 col:col + 28] = \
                band[ch].transpose(1, 0, 3, 2)                # [hin, G, dx, hout]
            tpar[base + 56, :, 0, col:col + 28] = \
                (t1[ch] + 3.0)[:, None]
    tpar = tpar.astype(f16)

    # pointwise weights [K=c(120), kc, mo, M=o(120)]
    pwT = pw_w.T                                              # [C, Cout]
    pwl = np.zeros((120, 2, 2, 120), f32)
    for kc in range(2):
        for mo in range(2):
            pwl[:, kc, mo, :] = pwT[kc * 120:(kc + 1) * 120,
                                    mo * 120:(mo + 1) * 120]
    pwl = pwl.astype(f16)

    s2 = bn2_g / np.sqrt(bn2_v + EPS)
    cblob = np.zeros((120, NCB), f32)
    # se1l [120, (kc, r)] = w1T[kc*120+p, r] / HW
    cblob[:, 0:120] = (se_w1.T / HW).reshape(2, 120, R).transpose(
        1, 0, 2).reshape(120, 120)
    cblob[:R, 120] = se_b1
    cblob[:R, 121:361] = se_w2.T.reshape(R, 240)
    cblob[:, 361:363] = (se_b2 + 3.0).reshape(2, 120).T
    cblob[:, 363:365] = s2.reshape(2, 120).T
    cblob[:, 365:367] = (s2 * pw_b).reshape(2, 120).T
    cblob[:, 367:369] = (bn2_b - bn2_m * s2).reshape(2, 120).T

    npad = CFG["trot"] * CFG["tch"]
    tpad = np.zeros((XP, npad, 2, KK, 56), f16)
    tpad[0:64, :, 0] = tpar[0:64, 0:npad].transpose(0, 1, 2, 3)
    tpad[57:121, :, 1] = tpar[57:121, 0:npad]
    shared = {
        "tpar": tpar, "pwl": pwl, "cblob": cblob, "tpad": tpad,
    }

    # x arena [XP, G, half, 36w, 4b]: rows base_k + 28*c_loc + h hold
    # x[4*half+b4, ch, h, j-2] (zero padded in w); row base_k+56 = 1.0
    x16 = x.astype(f16)
    in_maps = []
    for core in range(N_CORES):
        xc = x16[core * NB:(core + 1) * NB]                   # [NB, C, H, W]
        xh = xc.reshape(2, 4, C, H, W)                        # [half, b4, ...]
        xar = np.zeros((XP, G, 2, 36, 4), f16)
        for kb in range(2):
            base = 64 * kb
            for ci in range(2):
                ch = np.arange(G) * 4 + 2 * kb + ci
                # [half, b4, G, H, W] -> [H, G, half, W, b4]
                xar[base + 28 * ci:base + 28 * ci + 28, :, :, 2:2 + W, :] = \
                    xh[:, :, ch].transpose(3, 2, 0, 4, 1)
            xar[base + 56] = 1.0
        m = dict(shared)
        m["xar"] = xar
        in_maps.append(m)
    return in_maps


def kernel(**inputs):
    nc = build_nc()
    in_maps = prep_inputs(inputs)
    res = run_bass_kernel_spmd(nc, in_maps, list(range(N_CORES)))
    out = np.concatenate(
        [np.asarray(res.results[i]["y"]) for i in range(N_CORES)], axis=0)
    return out.astype(np.float32)


# revision 69
# speedup vs baseline: 1.0283x; 1.0108x over previous
"""Trainium2 Bass kernel for nn_DepthWiseSepConv (depthwise 5x5 + BN+hardswish
+ pointwise 1x1 + squeeze-excite gating + BN), data-parallel over batch on
8 NeuronCores.

Self-contained: hardcodes all shapes from the problem spec.

Per-core layout (B_loc = 8 images per core), all matmul operands fp16:

  - Depthwise conv, operand-swapped Toeplitz form: stationary lhsT = x
    [(2ch x 28h_in) + ones-row, (4b x 28w_out)], moving rhs = per-group
    block-diagonal Toeplitz [(2ch x 28h_in) + bias-row, (2ch x 28h_out)].
    Output lands pixel-major [(32w, 4b), (c4, h)] so only ONE transpose
    stage is needed (to channel-major) before the pointwise conv.  The x
    arena is packed (w-major, b-inner) so the stationary operand's free
    dims merge to a single walrus-legal dimension.  BN1 scale is folded
    into the Toeplitz weights; BN1 bias (+3 for the hardswish trick) rides
    an extra ones-row of x (dx=0 slice only).  5 dx shifts accumulate in
    PSUM.  Channel blocks sit at partition bases 0 and 64 (tile_position
    rule); the first rotation of Toeplitz slots is filled full-pad (zeros
    baked in DRAM), later refills overwrite only the payload columns.
  - Hardswish epilogue: ACT relu (PSUM->f16), DVE min, DVE (a-3)*m -> D.
  - One transpose stage (T2): [(4b,w), (g,c)@h] -> channel-major R tiles.
  - SE: per-image DVE reduces on R, two small matmuls, hardswish; gate and
    BN2 are folded into per-(image,half) ACT scale/bias vectors.
  - Pointwise: [120c x 120o] fp16 matmuls, N=392, single fused epilogue op.
"""

import sys

sys.path.insert(0, "/opt/trn_rl_repo")

import numpy as np

import concourse.bass as bass
import concourse.mybir as mybir
import concourse.tile as tile
from concourse import bacc
from concourse.bass_utils import run_bass_kernel_spmd
from concourse.masks import make_identity

# ---------------------------------------------------------------- constants
N_CORES = 8
B, C, H, W = 64, 240, 28, 28
NB = B // N_CORES           # images per core
KK = 5                      # depthwise kernel size
G = 60                      # groups of 4 channels
Cout = 240
R = 60                      # SE reduction dim
HW = H * W
EPS = 1e-5
NT = 392                    # pointwise free-dim tile (half an image)
XP = 121                    # arena partitions: block0 @0..57, block1 @64..121
NCB = 369                   # packed f32 const-blob columns

CFG = {
    "xchunks": (4, 10, 10, 12, 14, 10),  # group counts per x DMA chunk
    "tch": 4,               # groups per toeplitz rotation slot
    "trot": 4,              # toeplitz rotation depth
    "dwbufs": 4,            # DW psum pool depth
    "t2bufs": 4,            # T2 psum pool depth
    "copy_rr": "sv",        # T2 copy engine pattern (v=DVE, s=ACT)
    "d_rr": "sv",          # phase D epilogue engine pattern
    "stt_eng": "v",         # hardswish (a-3)*m engine
    "min_eng": "v",         # hardswish min engine
    "red_rr": "vs",         # SE reduce engines (v=DVE, s=ACT accum)
    "look": 14,             # input DMA lookahead (groups)
    "t2rate": 1,            # kc0 transpose quads interleaved per group
    "t2delay": 0,
    "pdbufs": 6,            # pointwise psum pool depth
}

F16 = mybir.dt.float16
F32 = mybir.dt.float32


# ---------------------------------------------------------------- builder
_BUILD_CACHE = {}


def build_nc(cfg_key=None):
    cfg = dict(CFG)
    if cfg_key is not None:
        cfg.update(cfg_key)
    key = tuple(sorted((k, str(v)) for k, v in cfg.items()))
    if key in _BUILD_CACHE:
        return _BUILD_CACHE[key]

    nc = bacc.Bacc("TRN2", target_bir_lowering=False, debug=False,
                   num_devices=N_CORES)

    xar_p = nc.declare_dram_parameter("xar", [XP, G, 2, 36, 4], F16,
                                      isOutput=False)
    tpar_p = nc.declare_dram_parameter("tpar", [XP, G, KK, 56], F16,
                                       isOutput=False)
    tpad_p = nc.declare_dram_parameter(
        "tpad", [XP, cfg["trot"] * cfg["tch"], 2, KK, 56], F16,
        isOutput=False)
    pwl_p = nc.declare_dram_parameter("pwl", [120, 2, 2, 120], F16,
                                      isOutput=False)
    cblob_p = nc.declare_dram_parameter("cblob", [120, NCB], F32,
                                        isOutput=False)
    y_p = nc.declare_dram_parameter("y", [NB, Cout, H, W], F16, isOutput=True)

    AL = mybir.AluOpType
    AF = mybir.ActivationFunctionType

    ENG = {"v": nc.vector, "s": nc.scalar, "g": nc.gpsimd}

    with tile.TileContext(nc) as tc:
        cst = tc.alloc_tile_pool(name="cst", bufs=1)
        pers = tc.alloc_tile_pool(name="pers", bufs=1)

        # ---- persistent arenas (chunked input DMAs for pipelined start)
        xchunks = list(cfg["xchunks"])
        assert sum(xchunks) == G
        xstarts = [sum(xchunks[:i]) for i in range(len(xchunks))]
        x_ch = [pers.tile([XP, n, 2, 36, 4], F16, name=f"xch{i}")
                for i, n in enumerate(xchunks)]
        xmap = {}
        for i, (s, n) in enumerate(zip(xstarts, xchunks)):
            for j in range(n):
                xmap[s + j] = (i, j)

        # toeplitz rotation slots: [121, TCH, blk, 5, 56]; the matmul rhs
        # view [:, g, :, dx, :] is the block-diagonal [121, 112].  Blk-major
        # keeps refill DMA runs at 560B (no small-transfer penalty).
        TCH, TROT = cfg["tch"], cfg["trot"]
        NTC = G // TCH
        t_sl = [pers.tile([XP, TCH, 2, KK, 56], F16, name=f"tsl{i}")
                for i in range(TROT)]

        def fill_toep(c):
            sl = t_sl[c % TROT]
            s = c * TCH
            if c < TROT:
                # first rotation: full-pad fill, zeros baked in DRAM
                nc.sync.dma_start(sl[:], tpad_p[:, s:s + TCH])
            else:
                # later refills only overwrite the payload block columns
                nc.sync.dma_start(sl[0:64, :, 0], tpar_p[0:64, s:s + TCH])
                nc.sync.dma_start(sl[57:121, :, 1],
                                  tpar_p[57:121, s:s + TCH])

        def issue_x(i):
            nc.sync.dma_start(x_ch[i][:],
                              xar_p[:, xstarts[i]:xstarts[i] + xchunks[i]])

        issue_x(0)
        fill_toep(0)
        issue_x(1)
        for i in range(1, TROT):
            fill_toep(i)

        # ---- constants in SBUF (single packed f32 blob + f16 pwl)
        pwl_sb = cst.tile([120, 2, 2, 120], F16)       # [K=c, kc, mo, M=o]
        nc.sync.dma_start(pwl_sb[:], pwl_p[:])
        cblob = cst.tile([120, NCB], F32)
        nc.sync.dma_start(cblob[:], cblob_p[:])
        se1l_sb = cblob[:, 0:120].rearrange("p (kc r) -> p kc r", kc=2)
        se1b_sb = cblob[0:R, 120:121]
        se2l_sb = cblob[0:R, 121:361].rearrange("p (mo o) -> p mo o", mo=2)
        se2b3_sb = cblob[:, 361:363]
        bn2s_sb = cblob[:, 363:365]
        bn2sb_sb = cblob[:, 365:367]
        bn2t_sb = cblob[:, 367:369]

        ident = cst.tile([128, 128], F16)
        make_identity(nc, ident[:])

        # remaining input DMAs are emitted inside the group loop, ordered
        # by the group that first needs them (see dma_sched)

        # D: depthwise+HS output, [(4b,w), half, g, c4, h]
        D = pers.tile([112, 2, G, 4, H], F16, name="dact")
        # R: channel-major pointwise rhs per kc chunk: [c, half, b4, h, w]
        Rt = [pers.tile([120, 2, 4, H, W], F16, name=f"rt{kc}")
              for kc in range(2)]
        s_sb = [pers.tile([120, NB], F32, name=f"s{kc}") for kc in range(2)]
        g_sb = [pers.tile([120, NB], F32, name=f"gate{mo}") for mo in range(2)]
        sc2 = [pers.tile([120, NB], F32, name=f"sc2{mo}") for mo in range(2)]
        bi2 = [pers.tile([120, NB], F32, name=f"bi2{mo}") for mo in range(2)]

        # input-DMA emission schedule: group -> [(fn, arg), ...], ordered by
        # first-need time with a few groups of lookahead
        LOOK = cfg.get("look", 12)
        dma_sched = {}
        for i in range(2, len(xchunks)):
            dma_sched.setdefault(max(0, xstarts[i] - LOOK), []).append(
                (issue_x, i))
        for c in range(TROT, NTC):
            dma_sched.setdefault((c - TROT) * TCH + TCH - 1, []).append(
                (fill_toep, c))

        # ================= Phase A: depthwise + BN1 + hardswish
        copy_rr = cfg["copy_rr"]
        cp_i = 0
        if True:
            pa = tc.alloc_tile_pool(name="pa", bufs=4)
            dwps = tc.alloc_tile_pool(name="dwps", bufs=cfg["dwbufs"],
                                      space="PSUM")
            t2ps = tc.alloc_tile_pool(name="t2ps", bufs=cfg["t2bufs"],
                                      space="PSUM", side="right")

            stt_eng = ENG[cfg["stt_eng"]]

            def do_group(g):
                ci, co = xmap[g]
                xg = x_ch[ci]
                tg, to = t_sl[(g // TCH) % TROT], g % TCH
                ps = dwps.tile([128, 2, 112], F32, tag="dw")
                n = 0
                for half in (0, 1):
                    for dx in range(KK):
                        # lhsT free (32w, 4b) strides (4,1) merges to (128,1)
                        nc.tensor.matmul(
                            ps[:, half, :],
                            xg[:, co, half, dx:dx + 32, :],
                            tg[:, to, :, dx, :],
                            start=(n == 0), stop=(n == 9))
                        n += 1
                a = pa.tile([112, 2, 112], F16, tag="a")
                nc.scalar.activation(a[:], ps[0:112], AF.Relu)
                m = pa.tile([112, 2, 112], F16, tag="m")
                ENG[cfg.get("min_eng", "v")].tensor_scalar(
                    m[:], a[:], 1.0 / 6.0, 1.0, AL.mult, AL.min)
                dst = D[:, :, g, :, :].rearrange("p half c h -> p half (c h)")
                stt_eng.scalar_tensor_tensor(dst, a[:], 3.0, m[:],
                                             AL.subtract, AL.mult)
                # emit upcoming input DMAs in the order they will be needed
                for fn, arg in dma_sched.get(g, ()):
                    fn(arg)

            red_rr = cfg["red_rr"]
            trash = pers.tile([120, HW], F16, name="trash")
            rd_i = 0

            def t2_quad(kc, half, h0):
                nonlocal cp_i
                tp = t2ps.tile([120, 4, 112], F16, tag="t2")
                for hi in range(4):
                    nc.tensor.transpose(
                        tp[:, hi, :],
                        D[:, half, 30 * kc:30 * kc + 30, :, h0 + hi],
                        ident[:112, :112])
                e = copy_rr[cp_i % len(copy_rr)]
                cp_i += 1
                cdst = Rt[kc][:, half, :, h0:h0 + 4, :]
                csrc = tp[:].rearrange("p hh (w b) -> p b hh w", b=4)
                if e == "s":
                    nc.scalar.copy(cdst, csrc)
                else:
                    ENG[e].tensor_copy(cdst, csrc)

            def t2_reduce(kc, half):
                nonlocal rd_i
                # SE partial reduces for this (kc, half) as soon as ready
                for b4 in range(4):
                    scol = s_sb[kc][:, 4 * half + b4:4 * half + b4 + 1]
                    rsrc = Rt[kc][:, half, b4].rearrange("p h w -> p (h w)")
                    e = red_rr[rd_i % len(red_rr)]
                    rd_i += 1
                    if e == "s":
                        nc.scalar.activation(trash[:], rsrc, AF.Identity,
                                             accum_out=scol)
                    else:
                        nc.vector.tensor_reduce(
                            scol, rsrc, mybir.AxisListType.X, AL.add)

            def do_t2(kc, half):
                for h0 in range(0, H, 4):
                    t2_quad(kc, half, h0)
                t2_reduce(kc, half)

            for g in range(30):
                do_group(g)
            # interleave the kc0 transpose quads with groups 30..: one quad
            # per group keeps the DW stream and input DMAs flowing
            t2q = [(0, half, h0) for half in (0, 1) for h0 in range(0, H, 4)]
            T2D = cfg.get("t2delay", 0)
            T2R = cfg.get("t2rate", 1)
            for g in range(30, 60):
                do_group(g)
                for k in range(T2R):
                    qi = (g - 30 - T2D) * T2R + k
                    if 0 <= qi < len(t2q):
                        t2_quad(*t2q[qi])
                        if qi == 6:
                            t2_reduce(0, 0)
                        elif qi == 13:
                            t2_reduce(0, 1)
            dwps.release()
            pa.release()

        # ================= Phase C + D, pipelined per image-half
        d_rr = cfg["d_rr"]
        d_i = 0
        rflat = [Rt[kc][:].rearrange("p half b h w -> p (half b h w)")
                 for kc in range(2)]
        # output staging: [120, half, b4, h, w] per mo; 1 DMA per (mo, half)
        ystage = [pers.tile([120, 2, 4, H, W], F16, name=f"ys{mo}")
                  for mo in range(2)]
        ysflat = [ystage[mo][:].rearrange("p half b h w -> p (half b h w)")
                  for mo in range(2)]
        seps = tc.alloc_tile_pool(name="seps", bufs=1, space="PSUM")
        pdps = None

        def se_chain(half):
            """gate + fused BN2 scale/bias for images 4*half..4*half+4"""
            hs = slice(4 * half, 4 * half + 4)
            ps1 = seps.tile([R, 4], F32, tag="se1")
            for kc in range(2):
                nc.tensor.matmul(ps1[:], se1l_sb[:, kc, :], s_sb[kc][:, hs],
                                 start=(kc == 0), stop=(kc == 1))
            h1 = pers.tile([R, 4], F32, name=f"h1_{half}")
            nc.scalar.activation(h1[:], ps1[:], AF.Relu, bias=se1b_sb)
            for mo in range(2):
                ps2 = seps.tile([120, 4], F32, tag="se2")
                nc.tensor.matmul(ps2[:], se2l_sb[:, mo, :], h1[:],
                                 start=True, stop=True)
                a2 = pers.tile([120, 4], F32, name=f"a2_{half}_{mo}")
                nc.scalar.activation(a2[:], ps2[:], AF.Relu,
                                     bias=se2b3_sb[:, mo:mo + 1])
                m2 = pers.tile([120, 4], F32, name=f"m2_{half}_{mo}")
                nc.vector.tensor_scalar(m2[:], a2[:], 1.0 / 6.0, 1.0,
                                        AL.mult, AL.min)
                nc.vector.scalar_tensor_tensor(g_sb[mo][:, hs], a2[:], 3.0,
                                               m2[:], AL.subtract, AL.mult)
                # scale2 = s2*g ; bias2 = (s2*pw_b)*g + t2  (per image col)
                nc.vector.tensor_tensor(
                    sc2[mo][:, hs], g_sb[mo][:, hs],
                    bn2s_sb[:, mo:mo + 1].to_broadcast((120, 4)), AL.mult)
                tmpb = pers.tile([120, 4], F32, name=f"tb_{half}_{mo}")
                nc.gpsimd.tensor_tensor(
                    tmpb[:], g_sb[mo][:, hs],
                    bn2sb_sb[:, mo:mo + 1].to_broadcast((120, 4)), AL.mult)
                nc.gpsimd.tensor_tensor(
                    bi2[mo][:, hs], tmpb[:],
                    bn2t_sb[:, mo:mo + 1].to_broadcast((120, 4)), AL.add)

        def do_d(half):
            nonlocal d_i
            for mo in range(2):
                for b in range(4 * half, 4 * half + 4):
                    for nt in range(2):
                        off = b * HW + nt * NT
                        ps = pdps.tile([120, NT], F32, tag="pw")
                        for kc in range(2):
                            nc.tensor.matmul(ps[:], pwl_sb[:, kc, mo, :],
                                             rflat[kc][:, off:off + NT],
                                             start=(kc == 0), stop=(kc == 1))
                        o = ysflat[mo][:, off:off + NT]
                        e = d_rr[d_i % len(d_rr)]
                        d_i += 1
                        if e == "s":
                            nc.scalar.activation(o, ps[:], AF.Identity,
                                                 bias=bi2[mo][:, b:b + 1],
                                                 scale=sc2[mo][:, b:b + 1])
                        else:
                            ENG[e].tensor_scalar(o, ps[:], sc2[mo][:, b:b + 1],
                                                 bi2[mo][:, b:b + 1],
                                                 AL.mult, AL.add)
                nyd = 4 if half == 1 else 2
                for bp in range(nyd):
                    w0 = 4 // nyd
                    b0 = 4 * half + w0 * bp
                    y_ap = y_p[b0:b0 + w0,
                               mo * 120:(mo + 1) * 120].rearrange(
                        "b c h w -> c b (h w)")
                    nc.sync.dma_start(
                        y_ap,
                        ystage[mo][:, half, w0 * bp:w0 * bp + w0].rearrange(
                            "p b h w -> p b (h w)"))

        if cfg.get("d_interleave"):
            pdps = tc.alloc_tile_pool(name="pdps", bufs=cfg["pdbufs"],
                                      space="PSUM")
            do_t2(1, 0)
            se_chain(0)
            do_d(0)
            do_t2(1, 1)
            se_chain(1)
            do_d(1)
            t2ps.release()
        else:
            do_t2(1, 0)
            do_t2(1, 1)
            t2ps.release()
            pdps = tc.alloc_tile_pool(name="pdps", bufs=cfg["pdbufs"],
                                      space="PSUM")
            se_chain(0)
            se_chain(1)
            do_d(0)
            do_d(1)

        pdps.release()
        seps.release()
        pers.release()
        cst.release()

    nc.compile()
    _BUILD_CACHE[key] = nc
    return nc


# ---------------------------------------------------------------- host prep
def prep_inputs(inputs, cfg_key=None):
    f32, f16 = np.float32, np.float16

    x = np.asarray(inputs["x"], f32)
    dw_w = np.asarray(inputs["dw_w"], f32)      # [C,1,5,5]
    dw_b = np.asarray(inputs["dw_b"], f32)
    bn1_g = np.asarray(inputs["bn1_g"], f32)
    bn1_b = np.asarray(inputs["bn1_b"], f32)
    bn1_m = np.asarray(inputs["bn1_m"], f32)
    bn1_v = np.asarray(inputs["bn1_v"], f32)
    pw_w = np.asarray(inputs["pw_w"], f32)      # [Cout, C]
    pw_b = np.asarray(inputs["pw_b"], f32)
    se_w1 = np.asarray(inputs["se_w1"], f32)    # [R, C]
    se_b1 = np.asarray(inputs["se_b1"], f32)
    se_w2 = np.asarray(inputs["se_w2"], f32)    # [Cout, R]
    se_b2 = np.asarray(inputs["se_b2"], f32)
    bn2_g = np.asarray(inputs["bn2_g"], f32)
    bn2_b = np.asarray(inputs["bn2_b"], f32)
    bn2_m = np.asarray(inputs["bn2_m"], f32)
    bn2_v = np.asarray(inputs["bn2_v"], f32)

    s1 = bn1_g / np.sqrt(bn1_v + EPS)
    t1 = s1 * (dw_b - bn1_m) + bn1_b

    # Compact Toeplitz [XP, G, KK, 56]: block kb rows base_k + 28*c_in + h_in
    # hold s1[ch]*w[ch, h_in-h_out+2, dx] at col 28*c_in + h_out; row
    # base_k+56 holds t1+3 (dx=0 only).  The device expands this to the
    # block-diagonal [121, 112] rhs via two column-offset DMAs per slot.
    hin = np.arange(H)[:, None]
    hout = np.arange(H)[None, :]
    Dh = hin - hout
    mask = np.abs(Dh) <= 2
    dyi = np.clip(Dh + 2, 0, 4)
    k = dw_w[:, 0] * s1[:, None, None]                        # [C, 5, 5]
    band = np.where(mask[None, :, :, None], k[:, dyi, :], 0.0)  # [C,hin,hout,dx]
    tpar = np.zeros((XP, G, KK, 56), f32)
    for kb in range(2):
        base = 64 * kb
        for ci in range(2):
            ch = np.arange(G) * 4 + 2 * kb + ci               # [G]
            col = 28 * ci
            tpar[base + 28 * ci:base + 28 * ci + 28, :, :,
                 col:col + 28] = \
                band[ch].transpose(1, 0, 3, 2)                # [hin, G, dx, hout]
            tpar[base + 56, :, 0, col:col + 28] = \
                (t1[ch] + 3.0)[:, None]
    tpar = tpar.astype(f16)

    # pointwise weights [K=c(120), kc, mo, M=o(120)]
    pwT = pw_w.T                                              # [C, Cout]
    pwl = np.zeros((120, 2, 2, 120), f32)
    for kc in range(2):
        for mo in range(2):
            pwl[:, kc, mo, :] = pwT[kc * 120:(kc + 1) * 120,
                                    mo * 120:(mo + 1) * 120]
    pwl = pwl.astype(f16)

    s2 = bn2_g / np.sqrt(bn2_v + EPS)
    cblob = np.zeros((120, NCB), f32)
    # se1l [120, (kc, r)] = w1T[kc*120+p, r] / HW
    cblob[:, 0:120] = (se_w1.T / HW).reshape(2, 120, R).transpose(
        1, 0, 2).reshape(120, 120)
    cblob[:R, 120] = se_b1
    cblob[:R, 121:361] = se_w2.T.reshape(R, 240)
    cblob[:, 361:363] = (se_b2 + 3.0).reshape(2, 120).T
    cblob[:, 363:365] = s2.reshape(2, 120).T
    cblob[:, 365:367] = (s2 * pw_b).reshape(2, 120).T
    cblob[:, 367:369] = (bn2_b - bn2_m * s2).reshape(2, 120).T

    npad = CFG["trot"] * CFG["tch"]
    tpad = np.zeros((XP, npad, 2, KK, 56), f16)
    tpad[0:64, :, 0] = tpar[0:64, 0:npad].transpose(0, 1, 2, 3)
    tpad[57:121, :, 1] = tpar[57:121, 0:npad]
    shared = {
        "tpar": tpar, "pwl": pwl, "cblob": cblob, "tpad": tpad,
    }

    # x arena [XP, G, half, 36w, 4b]: rows base_k + 28*c_loc + h hold
    # x[4*half+b4, ch, h, j-2] (zero padded in w); row base_k+56 = 1.0
    x16 = x.astype(f16)
    in_maps = []
    for core in range(N_CORES):
        xc = x16[core * NB:(core + 1) * NB]                   # [NB, C, H, W]
        xh = xc.reshape(2, 4, C, H, W)                        # [half, b4, ...]
        xar = np.zeros((XP, G, 2, 36, 4), f16)
        for kb in range(2):
            base = 64 * kb
            for ci in range(2):
                ch = np.arange(G) * 4 + 2 * kb + ci
                # [half, b4, G, H, W] -> [H, G, half, W, b4]
                xar[base + 28 * ci:base + 28 * ci + 28, :, :, 2:2 + W, :] = \
                    xh[:, :, ch].transpose(3, 2, 0, 4, 1)
            xar[base + 56] = 1.0
        m = dict(shared)
        m["xar"] = xar
        in_maps.append(m)
    return in_maps


def kernel(**inputs):
    nc = build_nc()
    in_maps = prep_inputs(inputs)
    res = run_bass_kernel_spmd(nc, in_maps, list(range(N_CORES)))
    out = np.concatenate(
        [np.asarray(res.results[i]["y"]) for i in range(N_CORES)], axis=0)
    return out.astype(np.float32)
